# revision 1
# baseline (speedup 1.0000x reference)
"""Multi-head attention (B=2, T=4096, D=512, H=8) on 8 Trainium2 cores.

Sharding: core i handles batch b=i//4, query rows q0=(i%4)*1024 .. q0+1024,
all 8 heads (full K/V of its batch computed on-core; no collectives).
Host pre-transposes x and weights so every DMA is contiguous, and rolls
x along T per core so each core's query block sits at columns 0:1024
(keys become a permutation of T, which attention is invariant to).

All matmuls run in float32r (TF32-like single-pass PE mode, ~1.5e-4 rel
err measured on HW). Softmax skips the max-subtraction (scores are
~N(0, 0.33); exp cannot overflow) and the row-sum comes free from a ones
column appended to V in the attn@V matmul (output partition 64).
"""
import sys
sys.path.insert(0, "/opt/trn_rl_repo")

import numpy as np
import concourse.bacc as bacc
import concourse.mybir as mybir
import concourse.tile as tile
from concourse.bass_utils import run_bass_kernel_spmd

F32 = mybir.dt.float32
F32R = mybir.dt.float32r
AF = mybir.ActivationFunctionType
MULT = mybir.AluOpType.mult

B, T, C = 2, 4096, 512
H, DK = 8, 64
TQ = 1024          # queries per core
NP = 4             # head pairs
KT = T // 128      # 32 k-tiles
CT = C // 128      # 4 contraction tiles

_cache = {}


def _build():
    nc = bacc.Bacc("TRN2")
    xbT = nc.declare_dram_parameter("xbT", [C, T], F32R, isOutput=False)
    wqT = nc.declare_dram_parameter("wqT", [C, C], F32R, isOutput=False)
    wkT = nc.declare_dram_parameter("wkT", [C, C], F32R, isOutput=False)
    wvT = nc.declare_dram_parameter("wvT", [C, C], F32R, isOutput=False)
    woT = nc.declare_dram_parameter("woT", [C, C], F32R, isOutput=False)
    # bias[:, 0] = bq/8, bias[:, 1] = bk, bias[:, 2] = bv  (col-block per pair)
    bias = nc.declare_dram_parameter("bias", [128, 3, NP], F32, isOutput=False)
    bo = nc.declare_dram_parameter("bo", [1, C], F32R, isOutput=False)
    # ind rows: 0 = head0 mask (1s in 0:64), 1 = head1 mask, 2 = all ones
    ind = nc.declare_dram_parameter("ind", [3, 128], F32R, isOutput=False)
    ones = nc.declare_dram_parameter("ones", [128, KT * 4], F32R, isOutput=False)
    out = nc.declare_dram_parameter("out", [TQ, C], F32, isOutput=True)

    with tile.TileContext(nc) as tc:
        attn_bufs, kt_bufs, big_bufs = 4, 2, 3
        use_prj, av_single = False, True
        with (
            tc.tile_pool(name="big", bufs=1) as bpool,
            tc.tile_pool(name="const", bufs=1) as cpool,
            tc.tile_pool(name="work", bufs=2) as wpool,
            tc.tile_pool(name="ktp", bufs=kt_bufs) as ktpool,
            tc.tile_pool(name="attnp", bufs=attn_bufs) as apool,
            tc.tile_pool(name="ps", bufs=big_bufs, space="PSUM") as ps,
            tc.tile_pool(name="prj", bufs=1, space="PSUM") as _psprj,
            tc.tile_pool(name="psav", bufs=1, space="PSUM") as psav,
        ):
            psprj = _psprj if use_prj else ps
            prjtag = "proj" if use_prj else "big"
            # ---- resident tensors ----
            xT = bpool.tile([128, CT, T], F32R, tag="xT")          # 64KB/part
            for ct in range(CT):
                for tch in range(4):
                    nc.sync.dma_start(
                        xT[:, ct, tch * 1024:(tch + 1) * 1024],
                        xbT[ct * 128:(ct + 1) * 128, tch * 1024:(tch + 1) * 1024])
            woTs = cpool.tile([128, CT, C], F32R, tag="woT")       # 8KB
            for ct in range(CT):
                nc.sync.dma_start(woTs[:, ct, :], woT[ct * 128:(ct + 1) * 128, :])
            bias_s = cpool.tile([128, 3, NP], F32, tag="bias")
            nc.sync.dma_start(bias_s[:], bias[:])
            # ind / bo live at partition 64 so matmul operand bases match the
            # rowsum row (PSUM partition 64) they pair with.
            inds = cpool.tile([65, 3, 128], F32R, tag="ind")
            nc.sync.dma_start(inds[64:65, :, :],
                              ind.rearrange("(o a) b -> o a b", o=1))
            bos = cpool.tile([65, C], F32R, tag="bo")
            nc.sync.dma_start(bos[64:65, :], bo[:])
            acat = bpool.tile([128, NP, TQ], F32R, tag="acat")     # 16KB

            # ---- V projection for one pair-group (2 pairs = 4 heads) ----
            # v2p[:, j*4 + pi*2 + h, 0:64] = V rows, col 64 = ones
            def v_proj(pg):
                v2p = bpool.tile([128, KT * 4, 65], F32R, tag="v2p")  # 33KB
                nc.sync.dma_start(v2p[:, :, 64], ones[:])
                wvs = bpool.tile([128, CT, 256], F32R, tag="wvs")
                for ct in range(CT):
                    nc.sync.dma_start(
                        wvs[:, ct, :],
                        wvT[ct * 128:(ct + 1) * 128, pg * 256:(pg + 1) * 256])
                for j in range(KT):
                    pv = psprj.tile([128, 1024], F32, tag=prjtag)
                    for ct in range(CT):
                        nc.tensor.matmul(
                            pv[:, 0:256],
                            xT[:, ct, j * 128:(j + 1) * 128],
                            wvs[:, ct, :],
                            start=(ct == 0), stop=(ct == CT - 1))
                    nc.vector.tensor_copy(
                        v2p[:, j * 4:(j + 1) * 4, 0:64],
                        pv[:, 0:256].rearrange("p (a b) -> p a b", b=64))
                return v2p

            def projs(p):
                # --- K^T projection: [128 d, 4096 t], bias bk per-partition ---
                kT = ktpool.tile([128, T], F32R, tag="kT")
                wks = bpool.tile([128, CT, 128], F32R, tag="wks")
                for ct in range(CT):
                    nc.sync.dma_start(
                        wks[:, ct, :],
                        wkT[ct * 128:(ct + 1) * 128, p * 128:(p + 1) * 128])
                for tp in range(4):
                    pk = psprj.tile([128, 1024], F32, tag=prjtag)
                    for half in range(2):
                        tch = tp * 2 + half
                        for ct in range(CT):
                            nc.tensor.matmul(
                                pk[:, half * 512:(half + 1) * 512],
                                wks[:, ct, :],
                                xT[:, ct, tch * 512:(tch + 1) * 512],
                                start=(ct == 0), stop=(ct == CT - 1))
                    nc.vector.tensor_scalar_add(
                        kT[:, tp * 1024:(tp + 1) * 1024], pk[:],
                        bias_s[:, 1, p:p + 1])
                # --- Q^T projection: [128 d, 1024 q], scale 1/8, bias bq/8 ---
                qT = ktpool.tile([128, TQ], F32R, tag="qT")
                wqs = bpool.tile([128, CT, 128], F32R, tag="wqs")
                for ct in range(CT):
                    nc.sync.dma_start(
                        wqs[:, ct, :],
                        wqT[ct * 128:(ct + 1) * 128, p * 128:(p + 1) * 128])
                pq = psprj.tile([128, 1024], F32, tag=prjtag)
                for half in range(2):
                    for ct in range(CT):
                        nc.tensor.matmul(
                            pq[:, half * 512:(half + 1) * 512],
                            wqs[:, ct, :],
                            xT[:, ct, half * 512:(half + 1) * 512],
                            start=(ct == 0), stop=(ct == CT - 1))
                nc.scalar.activation(qT[:], pq[:], AF.Identity,
                                     bias=bias_s[:, 0, p:p + 1], scale=0.125)
                return kT, qT

            def attn_pair(p, v2p, pi, kT, qT):
                # --- attention: scores^T -> exp -> attn@[V|1] accumulate ---
                stage = bpool.tile([65, 2, TQ], F32R, tag="stage")   # 8KB
                odd = wpool.tile([64, TQ], F32R, tag="sc")
                for h in range(2):
                    av = psav.tile([65, TQ], F32, tag="av" if av_single else f"av{h}")
                    d0 = h * 64
                    # pipeline unit = 3 half-tiles (1.5 k-tiles) to amortize
                    # the ACT per-instruction overhead; accumulation flags
                    # stay keyed on the k-tile index per 512-col PSUM region
                    Hh = 0
                    while Hh < 2 * KT:
                        n = min(2, 2 * KT - Hh)
                        pss = ps.tile([128, 512 * n], F32, tag="big")
                        for pos in range(n):
                            j, half = (Hh + pos) // 2, (Hh + pos) % 2
                            nc.tensor.matmul(
                                pss[:, pos * 512:(pos + 1) * 512],
                                kT[d0:d0 + 64, j * 128:(j + 1) * 128],
                                qT[d0:d0 + 64, half * 512:(half + 1) * 512],
                                start=True, stop=True)
                        at = apool.tile([128, 512 * n], F32R, tag="attn")
                        nc.scalar.activation(at[:], pss[:], AF.Exp)
                        for pos in range(n):
                            j, half = (Hh + pos) // 2, (Hh + pos) % 2
                            nc.tensor.matmul(
                                av[:, half * 512:(half + 1) * 512],
                                v2p[:, j * 4 + pi * 2 + h, :],
                                at[:, pos * 512:(pos + 1) * 512],
                                start=(j == 0), stop=(j == KT - 1))
                        Hh += n
                    # drain this head's accumulator so the next head can
                    # reuse the single PSUM slot
                    nc.vector.tensor_copy(stage[64:65, h, :], av[64:65, :])
                    if h == 0:
                        nc.vector.tensor_copy(acat[0:64, p, :], av[0:64, :])
                    else:
                        nc.vector.tensor_copy(odd[:], av[0:64, :])
                        nc.sync.dma_start(acat[64:128, p, :], odd[:])

                # --- normalize: U / rowsum + bv  into acat[:, p, :] ---
                pb = psprj.tile([128, 1024], F32, tag=prjtag)
                for half in range(2):
                    for h in range(2):
                        nc.tensor.matmul(
                            pb[:, half * 512:(half + 1) * 512],
                            inds[64:65, h, :],
                            stage[64:65, h, half * 512:(half + 1) * 512],
                            start=(h == 0), stop=(h == 1))
                rb = wpool.tile([128, TQ], F32, tag="sc")
                nc.vector.reciprocal(rb[:], pb[:])
                nc.vector.tensor_tensor(
                    out=acat[:, p, :], in0=acat[:, p, :], in1=rb[:], op=MULT)
                nc.vector.tensor_scalar_add(
                    acat[:, p, :], acat[:, p, :], bias_s[:, 2, p:p + 1])

            kq = projs(0)
            v2p = v_proj(0)
            attn_pair(0, v2p, 0, *kq)
            for p in range(1, NP):
                kq = projs(p)
                if p == 2:
                    v2p = v_proj(1)
                attn_pair(p, v2p, p % 2, *kq)

            # ---- output projection: out[t, :] = acat^T.T @ woT + bo ----
            for qt in range(8):
                po = psprj.tile([128, 1024], F32, tag=prjtag)
                for r in range(CT):
                    nc.tensor.matmul(
                        po[:, 0:512],
                        acat[:, r, qt * 128:(qt + 1) * 128],
                        woTs[:, r, :],
                        start=(r == 0), stop=False)
                nc.tensor.matmul(po[:, 0:512], inds[64:65, 2, :],
                                 bos[64:65, :], start=False, stop=True)
                ot = wpool.tile([128, C], F32, tag="sc")
                nc.vector.tensor_copy(ot[:], po[:, 0:512])
                nc.sync.dma_start(out[qt * 128:(qt + 1) * 128, :], ot[:])

    nc.compile()
    return nc


def _prep_inputs(x, Wq, bq, Wk, bk, Wv, bv, Wo, bo):
    wqT = np.ascontiguousarray(Wq.T)
    wkT = np.ascontiguousarray(Wk.T)
    wvT = np.ascontiguousarray(Wv.T)
    woT = np.ascontiguousarray(Wo.T)
    bias = np.stack([
        (bq / 8.0).reshape(NP, 128).T,
        bk.reshape(NP, 128).T,
        bv.reshape(NP, 128).T,
    ], axis=1).astype(np.float32)          # [128, 3, NP]
    bias = np.ascontiguousarray(bias)
    bo1 = np.ascontiguousarray(bo.reshape(1, C))
    ind = np.zeros((3, 128), np.float32)
    ind[0, 0:64] = 1.0
    ind[1, 64:128] = 1.0
    ind[2, :] = 1.0
    in_maps = []
    for i in range(8):
        b, q0 = i // 4, (i % 4) * TQ
        xbT = np.ascontiguousarray(np.roll(x[b].T, -q0, axis=1))
        in_maps.append({
            "xbT": xbT, "wqT": wqT, "wkT": wkT, "wvT": wvT, "woT": woT,
            "bias": bias, "bo": bo1, "ind": ind,
            "ones": np.ones((128, KT * 4), np.float32),
        })
    return in_maps


def kernel(x, Wq, bq, Wk, bk, Wv, bv, Wo, bo):
    x = np.asarray(x, np.float32)
    args = [np.asarray(a, np.float32) for a in
            (Wq, bq, Wk, bk, Wv, bv, Wo, bo)]
    if "nc" not in _cache:
        _cache["nc"] = _build()
    nc = _cache["nc"]
    in_maps = _prep_inputs(x, *args)
    res = run_bass_kernel_spmd(nc, in_maps, list(range(8)))
    outf = np.empty((B, T, C), np.float32)
    for i in range(8):
        b, q0 = i // 4, (i % 4) * TQ
        outf[b, q0:q0 + TQ, :] = res.results[i]["out"]
    return outf



# revision 2
# speedup vs baseline: 1.0148x; 1.0148x over previous
"""Multi-head attention (B=2, T=4096, D=512, H=8) on 8 Trainium2 cores — v2.

Sharding: core i = batch i//4, query rows (i%4)*1024..+1024, all heads.
Host rolls x along T so each core's queries sit at t=0:1024.

Engine plan (per core):
- PE: bf16 projections (Q,K,V,O); fp8e4m3 DoubleRow for scores (d_k split
  32+32 into DR subtiles, K/Q stored "folded" [32p, 2, t] via host-permuted
  weight columns) and for attn@V (k-tile pairs are natural DR subtiles).
- Softmax exp is the bottleneck: split across ACT (true Exp -> fp8 out) and
  DVE (Schraudolph bit-trick: round(a*s+b) as int8, bitcast to fp8e4m3).
- Rowsum via ones column in V; normalize on DVE with stride-0 broadcast.
- attn^T via PE transpose (bf16, identity), O proj bf16, bias via 1-row
  matmul. b_k dropped (softmax-invariant), b_v folded into b_o host-side.
Emission interleaves A@V halves and g1 projections inside the next head's
scores/exp stream so the in-order engine queues never starve.
"""
import sys
sys.path.insert(0, "/opt/trn_rl_repo")

import numpy as np
import ml_dtypes
import concourse.bacc as bacc
import concourse.mybir as mybir
import concourse.tile as tile
from concourse.bass_utils import run_bass_kernel_spmd

F32 = mybir.dt.float32
BF16 = mybir.dt.bfloat16
F8 = mybir.dt.float8e4
I8 = mybir.dt.int8
F32R = mybir.dt.float32r
AF = mybir.ActivationFunctionType
ALU = mybir.AluOpType
DR = mybir.MatmulPerfMode.DoubleRow

f8t = ml_dtypes.float8_e4m3
bft = ml_dtypes.bfloat16

B, T, C = 2, 4096, 512
H, DK, TQ = 8, 64, 1024
CT, KT, QT = 4, 32, 8

EXP_SCALE = 0.25                      # q scaled x4 host-side; /8 net
SCH_A = (8.0 / np.log(2.0)) * EXP_SCALE
SCH_B = 56.0                          # hw rounds half-away; bias 7*8
ACT_N = 17                            # exp tiles on ACT per 32

_cache = {}


def _build():
    nc = bacc.Bacc("TRN2")
    xbT = nc.declare_dram_parameter("xbT", [C, T], BF16, isOutput=False)
    wkb = nc.declare_dram_parameter("wkb", [128, CT * 512], BF16, isOutput=False)
    wqb = nc.declare_dram_parameter("wqb", [128, CT * 512], BF16, isOutput=False)
    wvb = nc.declare_dram_parameter("wvb", [128, CT * 512], BF16, isOutput=False)
    wob = nc.declare_dram_parameter("wob", [128, 4 * 512], BF16, isOutput=False)
    bqs = nc.declare_dram_parameter("bqs", [128, 4], F32, isOutput=False)
    bo1 = nc.declare_dram_parameter("bo1", [1, 512], F32R, isOutput=False)
    one1 = nc.declare_dram_parameter("one1", [1, 128], F32R, isOutput=False)
    idb = nc.declare_dram_parameter("idb", [128, 128], BF16, isOutput=False)
    out = nc.declare_dram_parameter("out", [TQ, C], F32, isOutput=True)

    cvt_toggle = [0]
    ob_toggle = [0]

    with tile.TileContext(nc) as tc:
        with (
            tc.tile_pool(name="const", bufs=1) as cpool,
            tc.tile_pool(name="sb", bufs=1) as sb,
            tc.tile_pool(name="apool", bufs=2) as apool,
            tc.tile_pool(name="atp", bufs=2) as atp,
            tc.tile_pool(name="outp", bufs=2) as outp,
            tc.tile_pool(name="rp", bufs=2) as rp,
            tc.tile_pool(name="big", bufs=3, space="PSUM") as big,
            tc.tile_pool(name="small", bufs=2, space="PSUM") as small,
        ):
            # ---------- x queries chunk + early consts first ----------
            xT = sb.tile([128, CT, T], BF16, tag="xT")
            for ct in range(CT):
                nc.sync.dma_start(xT[:, ct, 0:1024], xbT[ct * 128:(ct + 1) * 128, 0:1024])
            wq = cpool.tile([128, CT, 2, 2, 128], BF16, tag="wq")
            nc.scalar.dma_start(
                wq[:], wqb.rearrange("p (a b c d) -> p a b c d", a=CT, b=2, c=2))
            wk = cpool.tile([128, CT, 2, 2, 128], BF16, tag="wk")
            nc.scalar.dma_start(
                wk[:], wkb.rearrange("p (a b c d) -> p a b c d", a=CT, b=2, c=2))
            bq = cpool.tile([128, 4], F32, tag="bq")
            nc.scalar.dma_start(bq[:], bqs[:])
            wv = cpool.tile([128, CT, 512], BF16, tag="wv")
            nc.scalar.dma_start(wv[:], wvb.rearrange("p (a b) -> p a b", a=CT))
            ident = cpool.tile([128, 128], BF16, tag="ident")
            nc.scalar.dma_start(ident[:], idb[:])
            ones = cpool.tile([1, 128], F32R, tag="ones")
            nc.scalar.dma_start(ones[:], one1[:])
            bos = cpool.tile([1, 512], F32R, tag="bos")
            nc.scalar.dma_start(bos[:], bo1[:])
            wo = cpool.tile([128, 4, 512], BF16, tag="wo")
            nc.sync.dma_start(wo[:], wob.rearrange("p (a b) -> p a b", a=4))
            for tch in range(1, 4):
                for ct in range(CT):
                    nc.sync.dma_start(
                        xT[:, ct, tch * 1024:(tch + 1) * 1024],
                        xbT[ct * 128:(ct + 1) * 128, tch * 1024:(tch + 1) * 1024])

            # folded fp8 K^T/Q^T: [p = 4heads x 32dsub, g, plane, t]
            kfold = sb.tile([128, 2, 2, T], F8, tag="kfold")
            qfold = sb.tile([128, 2, 2, TQ], F8, tag="qfold")
            # V: [p=k-within-tile, d(65: row 64=ones), j, g, hi]
            vsb = sb.tile([128, 65, KT, 2, 4], F8, tag="vsb")
            nc.vector.memset(vsb[:, 64, :, :, :], 1.0)
            # normalized attention, [p=q-within-tile, qt, h, d]
            attn = sb.tile([128, QT, H, DK], BF16, tag="attn")

            # ---------- converts (psum f32 -> sbuf fp8) ----------
            def convert(dst, src, bias=None):
                eng = 0 if cvt_toggle[0] % 3 != 2 else 1   # 2/3 on ACT
                cvt_toggle[0] += 1
                if eng == 0:
                    if bias is not None:
                        nc.scalar.activation(dst, src, AF.Identity, bias=bias)
                    else:
                        nc.scalar.copy(dst, src)
                else:
                    if bias is not None:
                        nc.vector.tensor_scalar_add(dst, src, bias)
                    else:
                        nc.vector.tensor_copy(dst, src)

            # ---------- projections (512-wide pieces, small psum ring) ----------
            def q_piece(g, s, half):
                pq = small.tile([128, 512], F32, tag="sm")
                for ct in range(CT):
                    nc.tensor.matmul(
                        pq[:], wq[:, ct, g, s, :],
                        xT[:, ct, half * 512:(half + 1) * 512],
                        start=(ct == 0), stop=(ct == CT - 1))
                convert(qfold[:, g, s, half * 512:(half + 1) * 512], pq[:],
                        bias=bq[:, g * 2 + s:g * 2 + s + 1])

            def k_piece(g, tch, s, half):
                pk = small.tile([128, 512], F32, tag="sm")
                c0 = tch * 1024 + half * 512
                for ct in range(CT):
                    nc.tensor.matmul(
                        pk[:], wk[:, ct, g, s, :], xT[:, ct, c0:c0 + 512],
                        start=(ct == 0), stop=(ct == CT - 1))
                convert(kfold[:, g, s, c0:c0 + 512], pk[:])

            def v_piece(g, jq2):
                # two k-tiles (j = 2*jq2, +1)
                pv = small.tile([128, 2, 256], F32, tag="sm")
                for jj in range(2):
                    j = jq2 * 2 + jj
                    for ct in range(CT):
                        nc.tensor.matmul(
                            pv[:, jj, :],
                            xT[:, ct, j * 128:(j + 1) * 128],
                            wv[:, ct, g * 256:(g + 1) * 256],
                            start=(ct == 0), stop=(ct == CT - 1))
                convert(
                    vsb[:, 0:64, jq2 * 2:(jq2 + 1) * 2, g, :],
                    pv[:].rearrange("p a (c d) -> p d a c", c=4))

            # ---------- attention ----------
            def scores_head(h, extras):
                g, hi = divmod(h, 4)
                base = 32 * hi
                A = apool.tile([128, KT, TQ], F8, tag="A")
                for j in range(KT):
                    pss = big.tile([128, 1024], F32, tag="pp")
                    for half in range(2):
                        nc.tensor.matmul(
                            pss[:, half * 512:(half + 1) * 512],
                            kfold[base:base + 32, g, :, j * 128:(j + 1) * 128],
                            qfold[base:base + 32, g, :, half * 512:(half + 1) * 512],
                            start=True, stop=True, perf_mode=DR,
                            tile_position=(base, 0))
                    if (j * ACT_N) % 32 < ACT_N:
                        nc.scalar.activation(A[:, j, :], pss[:], AF.Exp,
                                             scale=EXP_SCALE)
                    else:
                        nc.vector.tensor_scalar(A[:, j, :].bitcast(I8), pss[:],
                                                SCH_A, SCH_B, ALU.mult, ALU.add)
                    if j in extras:
                        extras[j]()
                return A

            def av_half(h, A, qh):
                g, hi = divmod(h, 4)
                av = small.tile([128, 4, 65], F32, tag="sm")
                for qq in range(4):
                    qt = qh * 4 + qq
                    for jp in range(16):
                        nc.tensor.matmul(
                            av[:, qq, :],
                            A[:, 2 * jp:2 * jp + 2, qt * 128:(qt + 1) * 128],
                            vsb[:, :, 2 * jp:2 * jp + 2, g, hi].rearrange(
                                "p d a -> p a d"),
                            start=(jp == 0), stop=(jp == 15), perf_mode=DR)
                ri = rp.tile([128, 4], F32, tag="ri")
                nc.vector.reciprocal(ri[:], av[:, :, 64])
                nc.vector.tensor_tensor(
                    out=attn[:, qh * 4:(qh + 1) * 4, h, :],
                    in0=av[:, :, 0:64],
                    in1=ri[:][:, :, None].broadcast_to([128, 4, 64]),
                    op=ALU.mult)

            # ---------- output projection (batched per qt-half) ----------
            def tail_half(qh):
                pt = big.tile([128, 16, 128], BF16, tag="pp")
                for qq in range(4):
                    qt = qh * 4 + qq
                    for c in range(4):
                        nc.tensor.matmul(
                            pt[:, qq * 4 + c, :],
                            attn[:, qt, 2 * c:2 * c + 2, :],
                            ident[:],
                            is_transpose=True)
                aT = atp.tile([128, 16, 128], BF16, tag="aT")
                nc.vector.tensor_copy(aT[:], pt[:])
                for pair in range(2):
                    po = big.tile([128, 2, 512], F32, tag="pp")
                    for i in range(2):
                        qq = pair * 2 + i
                        for c in range(4):
                            nc.tensor.matmul(po[:, i, :], aT[:, qq * 4 + c, :],
                                             wo[:, c, :],
                                             start=(c == 0), stop=False)
                        nc.tensor.matmul(po[:, i, :], ones[:], bos[:],
                                         start=False, stop=True)
                    ob = outp.tile([128, 2, 512], F32, tag="ob")
                    if ob_toggle[0] % 2 == 0:
                        nc.scalar.copy(ob[:], po[:])
                    else:
                        nc.vector.tensor_copy(ob[:], po[:])
                    ob_toggle[0] += 1
                    for i in range(2):
                        qt = qh * 4 + pair * 2 + i
                        nc.sync.dma_start(out[qt * 128:(qt + 1) * 128, :],
                                          ob[:, i, :])

            # ---------- emission schedule ----------
            def mk(fn, *a):
                return lambda: fn(*a)

            # startup: enough of Q/K(g0) to begin head 0
            for s in range(2):
                for half in range(2):
                    q_piece(0, s, half)
            for s in range(2):
                for half in range(2):
                    k_piece(0, 0, s, half)

            work = []
            work += [mk(k_piece, 0, tch, s, half)
                     for tch in range(1, 4) for s in range(2) for half in range(2)]
            work += [mk(v_piece, 0, i) for i in range(16)]
            work += [mk(k_piece, 1, tch, s, half)
                     for tch in range(4) for s in range(2) for half in range(2)]
            work += [mk(q_piece, 1, s, half) for s in range(2) for half in range(2)]
            work += [mk(v_piece, 1, i) for i in range(16)]
            work.reverse()   # pop() from the end

            A_cur = scores_head(0, {(2 * i + 1): work.pop()
                                    for i in range(16) if work})
            for h in range(H):
                if h + 1 < H:
                    ex = {}
                    Ah = A_cur
                    for i in range(16):
                        if i == 12:
                            ex[2 * i + 1] = mk(av_half, h, Ah, 0)
                        elif i == 14:
                            ex[2 * i + 1] = mk(av_half, h, Ah, 1)
                        elif work:
                            ex[2 * i + 1] = work.pop()
                    A_cur = scores_head(h + 1, ex)
                else:
                    av_half(h, A_cur, 0)
                    tail_half(0)
                    av_half(h, A_cur, 1)
                    tail_half(1)

    nc.compile()
    return nc


def _prep_consts(Wq, bq, Wk, bk, Wv, bv, Wo, bo):
    # wk/wq: [p, ct, g, s, c0]; D = (g*4 + c0//32)*64 + s*32 + c0%32
    def fold_w(W):
        wf = np.empty((128, CT, 2, 2, 128), np.float32)
        c0 = np.arange(128)
        for g in range(2):
            for s in range(2):
                D = (g * 4 + c0 // 32) * 64 + s * 32 + c0 % 32
                for ct in range(CT):
                    wf[:, ct, g, s, :] = W[D, ct * 128:(ct + 1) * 128].T
        return wf.reshape(128, CT * 512).astype(bft)

    wkb = fold_w(Wk)
    wqb = fold_w(0.5 * Wq)
    # wv: [p, ct, col = g*256 + hi*64 + d] = Wv[(g*4+hi)*64+d, ct*128+p]
    wvb = np.ascontiguousarray(
        Wv.T.reshape(CT, 128, 512).transpose(1, 0, 2)
    ).reshape(128, CT * 512).astype(bft)
    # wo: [p, c, C0] = Wo[C0, c*128+p]
    wob = np.ascontiguousarray(
        Wo.T.reshape(4, 128, 512).transpose(1, 0, 2)
    ).reshape(128, 4 * 512).astype(bft)
    # bq: [p, g*2+s] = 0.5*bq[(g*4+p//32)*64 + s*32 + p%32]
    bqa = np.empty((128, 4), np.float32)
    p = np.arange(128)
    for g in range(2):
        for s in range(2):
            bqa[:, g * 2 + s] = 0.5 * bq[(g * 4 + p // 32) * 64 + s * 32 + p % 32]
    bo1 = (bo + Wo @ bv).reshape(1, 512).astype(np.float32)
    one1 = np.ones((1, 128), np.float32)
    idb = np.eye(128, dtype=np.float32).astype(bft)
    return {"wkb": wkb, "wqb": wqb, "wvb": wvb, "wob": wob,
            "bqs": bqa, "bo1": bo1, "one1": one1, "idb": idb}


def kernel(x, Wq, bq, Wk, bk, Wv, bv, Wo, bo):
    x = np.asarray(x, np.float32)
    consts = _prep_consts(*[np.asarray(a, np.float32) for a in
                            (Wq, bq, Wk, bk, Wv, bv, Wo, bo)])
    if "nc" not in _cache:
        _cache["nc"] = _build()
    nc = _cache["nc"]
    in_maps = []
    for i in range(8):
        b, q0 = i // 4, (i % 4) * TQ
        xbT = np.ascontiguousarray(np.roll(x[b], -q0, axis=0).T).astype(bft)
        m = {"xbT": xbT}
        m.update(consts)
        in_maps.append(m)
    res = run_bass_kernel_spmd(nc, in_maps, list(range(8)))
    outf = np.empty((B, T, C), np.float32)
    for i in range(8):
        b, q0 = i // 4, (i % 4) * TQ
        outf[b, q0:q0 + TQ, :] = res.results[i]["out"]
    return outf


# revision 4
# speedup vs baseline: 1.0282x; 1.0132x over previous
"""Multi-head attention (B=2, T=4096, D=512, H=8) on 8 Trainium2 cores — v2.

Sharding: core i = batch i//4, query rows (i%4)*1024..+1024, all heads.
Host rolls x along T so each core's queries sit at t=0:1024.

Engine plan (per core):
- PE: bf16 projections (Q,K,V,O); fp8e4m3 DoubleRow for scores (d_k split
  32+32 into DR subtiles, K/Q stored "folded" [32p, 2, t] via host-permuted
  weight columns) and for attn@V (k-tile pairs are natural DR subtiles).
- Softmax exp is the bottleneck: split across ACT (true Exp -> fp8 out) and
  DVE (Schraudolph bit-trick: round(a*s+b) as int8, bitcast to fp8e4m3).
- Rowsum via ones column in V; normalize on DVE with stride-0 broadcast.
- attn^T via PE transpose (bf16, identity), O proj bf16, bias via 1-row
  matmul. b_k dropped (softmax-invariant), b_v folded into b_o host-side.
Emission interleaves A@V halves and g1 projections inside the next head's
scores/exp stream so the in-order engine queues never starve.
"""
import sys
sys.path.insert(0, "/opt/trn_rl_repo")

import numpy as np
import ml_dtypes
import concourse.bacc as bacc
import concourse.mybir as mybir
import concourse.tile as tile
from concourse.bass_utils import run_bass_kernel_spmd

F32 = mybir.dt.float32
BF16 = mybir.dt.bfloat16
F8 = mybir.dt.float8e4
I8 = mybir.dt.int8
F32R = mybir.dt.float32r
AF = mybir.ActivationFunctionType
ALU = mybir.AluOpType
DR = mybir.MatmulPerfMode.DoubleRow

f8t = ml_dtypes.float8_e4m3
bft = ml_dtypes.bfloat16

B, T, C = 2, 4096, 512
H, DK, TQ = 8, 64, 1024
CT, KT, QT = 4, 32, 8

EXP_SCALE = 0.25                      # q scaled x4 host-side; /8 net
SCH_A = (8.0 / np.log(2.0)) * EXP_SCALE
SCH_B = 56.0                          # hw rounds half-away; bias 7*8
ACT_N = 17                            # exp tiles on ACT per 32

_cache = {}


def _build():
    nc = bacc.Bacc("TRN2")
    xbT = nc.declare_dram_parameter("xbT", [C, T], BF16, isOutput=False)
    wkb = nc.declare_dram_parameter("wkb", [128, CT * 512], BF16, isOutput=False)
    wqb = nc.declare_dram_parameter("wqb", [128, CT * 512], BF16, isOutput=False)
    wvb = nc.declare_dram_parameter("wvb", [128, CT * 512], BF16, isOutput=False)
    wob = nc.declare_dram_parameter("wob", [128, 4 * 512], BF16, isOutput=False)
    bqs = nc.declare_dram_parameter("bqs", [128, 4], F32, isOutput=False)
    bo1 = nc.declare_dram_parameter("bo1", [1, 512], F32R, isOutput=False)
    one1 = nc.declare_dram_parameter("one1", [1, 128], F32R, isOutput=False)
    idb = nc.declare_dram_parameter("idb", [128, 128], BF16, isOutput=False)
    out = nc.declare_dram_parameter("out", [TQ, C], F32, isOutput=True)

    cvt_toggle = [0]
    ob_toggle = [0]

    with tile.TileContext(nc) as tc:
        with (
            tc.tile_pool(name="const", bufs=1) as cpool,
            tc.tile_pool(name="sb", bufs=1) as sb,
            tc.tile_pool(name="apool", bufs=2) as apool,
            tc.tile_pool(name="atp", bufs=2) as atp,
            tc.tile_pool(name="outp", bufs=2) as outp,
            tc.tile_pool(name="rp", bufs=2) as rp,
            tc.tile_pool(name="big", bufs=3, space="PSUM") as big,
            tc.tile_pool(name="small", bufs=2, space="PSUM") as small,
        ):
            # ---------- x queries chunk + early consts first ----------
            xT = sb.tile([128, CT, T], BF16, tag="xT")
            for ct in range(CT):
                nc.sync.dma_start(xT[:, ct, 0:1024], xbT[ct * 128:(ct + 1) * 128, 0:1024])
            wq = cpool.tile([128, CT, 2, 2, 128], BF16, tag="wq")
            nc.scalar.dma_start(
                wq[:], wqb.rearrange("p (a b c d) -> p a b c d", a=CT, b=2, c=2))
            wk = cpool.tile([128, CT, 2, 2, 128], BF16, tag="wk")
            nc.scalar.dma_start(
                wk[:], wkb.rearrange("p (a b c d) -> p a b c d", a=CT, b=2, c=2))
            bq = cpool.tile([128, 4], F32, tag="bq")
            nc.scalar.dma_start(bq[:], bqs[:])
            wv = cpool.tile([128, CT, 512], BF16, tag="wv")
            nc.sync.dma_start(wv[:], wvb.rearrange("p (a b) -> p a b", a=CT))
            ident = cpool.tile([128, 128], BF16, tag="ident")
            nc.sync.dma_start(ident[:], idb[:])
            ones = cpool.tile([1, 128], F32R, tag="ones")
            nc.sync.dma_start(ones[:], one1[:])
            bos = cpool.tile([1, 512], F32R, tag="bos")
            nc.sync.dma_start(bos[:], bo1[:])
            wo = cpool.tile([128, 4, 512], BF16, tag="wo")
            nc.sync.dma_start(wo[:], wob.rearrange("p (a b) -> p a b", a=4))
            for tch in range(1, 4):
                for ct in range(CT):
                    nc.sync.dma_start(
                        xT[:, ct, tch * 1024:(tch + 1) * 1024],
                        xbT[ct * 128:(ct + 1) * 128, tch * 1024:(tch + 1) * 1024])

            # folded fp8 K^T/Q^T: [p = 4heads x 32dsub, g, plane, t]
            kfold = sb.tile([128, 2, 2, T], F8, tag="kfold")
            qfold = sb.tile([128, 2, 2, TQ], F8, tag="qfold")
            # V: [p=k-within-tile, d(65: row 64=ones), j, g, hi]
            vsb = sb.tile([128, 65, KT, 2, 4], F8, tag="vsb")
            nc.vector.memset(vsb[:, 64, :, :, :], 1.0)
            # normalized attention, [p=q-within-tile, qt, h, d]
            attn = sb.tile([128, QT, H, DK], BF16, tag="attn")

            # ---------- converts (psum f32 -> sbuf fp8) ----------
            def convert(dst, src, bias=None):
                eng = 0 if cvt_toggle[0] % 5 < 3 else 1   # 3/5 on ACT
                cvt_toggle[0] += 1
                if eng == 0:
                    if bias is not None:
                        nc.scalar.activation(dst, src, AF.Identity, bias=bias)
                    else:
                        nc.scalar.copy(dst, src)
                else:
                    if bias is not None:
                        nc.vector.tensor_scalar_add(dst, src, bias)
                    else:
                        nc.vector.tensor_copy(dst, src)

            # ---------- projections (512-wide pieces, small psum ring) ----------
            def q_piece(g, s, half):
                pq = small.tile([128, 512], F32, tag="sm")
                for ct in range(CT):
                    nc.tensor.matmul(
                        pq[:], wq[:, ct, g, s, :],
                        xT[:, ct, half * 512:(half + 1) * 512],
                        start=(ct == 0), stop=(ct == CT - 1))
                convert(qfold[:, g, s, half * 512:(half + 1) * 512], pq[:],
                        bias=bq[:, g * 2 + s:g * 2 + s + 1])

            def k_piece(g, tch, s, half):
                pk = small.tile([128, 512], F32, tag="sm")
                c0 = tch * 1024 + half * 512
                for ct in range(CT):
                    nc.tensor.matmul(
                        pk[:], wk[:, ct, g, s, :], xT[:, ct, c0:c0 + 512],
                        start=(ct == 0), stop=(ct == CT - 1))
                convert(kfold[:, g, s, c0:c0 + 512], pk[:])

            def v_piece(g, jq2):
                # two k-tiles (j = 2*jq2, +1)
                pv = small.tile([128, 2, 256], F32, tag="sm")
                for jj in range(2):
                    j = jq2 * 2 + jj
                    for ct in range(CT):
                        nc.tensor.matmul(
                            pv[:, jj, :],
                            xT[:, ct, j * 128:(j + 1) * 128],
                            wv[:, ct, g * 256:(g + 1) * 256],
                            start=(ct == 0), stop=(ct == CT - 1))
                convert(
                    vsb[:, 0:64, jq2 * 2:(jq2 + 1) * 2, g, :],
                    pv[:].rearrange("p a (c d) -> p d a c", c=4))

            # ---------- attention ----------
            def scores_head(h, extras):
                g, hi = divmod(h, 4)
                base = 32 * hi
                A = apool.tile([128, KT, TQ], F8, tag="A")
                for j in range(KT):
                    pss = big.tile([128, 1024], F32, tag="pp")
                    for half in range(2):
                        nc.tensor.matmul(
                            pss[:, half * 512:(half + 1) * 512],
                            kfold[base:base + 32, g, :, j * 128:(j + 1) * 128],
                            qfold[base:base + 32, g, :, half * 512:(half + 1) * 512],
                            start=True, stop=True, perf_mode=DR,
                            tile_position=(base, 0))
                    if (j * ACT_N) % 32 < ACT_N:
                        nc.scalar.activation(A[:, j, :], pss[:], AF.Exp,
                                             scale=EXP_SCALE)
                    else:
                        nc.vector.tensor_scalar(A[:, j, :].bitcast(I8), pss[:],
                                                SCH_A, SCH_B, ALU.mult, ALU.add)
                    if j in extras:
                        extras[j]()
                return A

            def av_half(h, A, qh):
                g, hi = divmod(h, 4)
                av = small.tile([128, 4, 65], F32, tag="sm")
                for qq in range(4):
                    qt = qh * 4 + qq
                    for jp in range(16):
                        nc.tensor.matmul(
                            av[:, qq, :],
                            A[:, 2 * jp:2 * jp + 2, qt * 128:(qt + 1) * 128],
                            vsb[:, :, 2 * jp:2 * jp + 2, g, hi].rearrange(
                                "p d a -> p a d"),
                            start=(jp == 0), stop=(jp == 15), perf_mode=DR)
                ri = rp.tile([128, 4], F32, tag="ri")
                nc.vector.reciprocal(ri[:], av[:, :, 64])
                nc.vector.tensor_tensor(
                    out=attn[:, qh * 4:(qh + 1) * 4, h, :],
                    in0=av[:, :, 0:64],
                    in1=ri[:][:, :, None].broadcast_to([128, 4, 64]),
                    op=ALU.mult)

            # ---------- output projection ----------
            # attn^T accumulates in SBUF per head-pair as soon as both heads
            # are normalized; the tail is only the last pair + O-proj chains.
            aTsb = sb.tile([128, 4, QT, 128], BF16, tag="aTsb")

            def hp_transpose(c, halves=(0, 1)):
                for hf in halves:
                    pt = small.tile([128, 4, 128], BF16, tag="sm")
                    for qq in range(4):
                        qt = hf * 4 + qq
                        nc.tensor.matmul(
                            pt[:, qq, :],
                            attn[:, qt, 2 * c:2 * c + 2, :],
                            ident[:],
                            is_transpose=True)
                    nc.vector.tensor_copy(aTsb[:, c, hf * 4:(hf + 1) * 4, :], pt[:])

            def tail_pair(pair):
                po = big.tile([128, 2, 512], F32, tag="pp")
                for i in range(2):
                    qt = 2 * pair + i
                    for c in range(4):
                        nc.tensor.matmul(po[:, i, :], aTsb[:, c, qt, :],
                                         wo[:, c, :],
                                         start=(c == 0), stop=False)
                    nc.tensor.matmul(po[:, i, :], ones[:], bos[:],
                                     start=False, stop=True)
                ob = outp.tile([128, 2, 512], F32, tag="ob")
                if ob_toggle[0] % 2 == 0:
                    nc.scalar.copy(ob[:], po[:])
                else:
                    nc.vector.tensor_copy(ob[:], po[:])
                ob_toggle[0] += 1
                for i in range(2):
                    qt = 2 * pair + i
                    nc.sync.dma_start(out[qt * 128:(qt + 1) * 128, :],
                                      ob[:, i, :])

            # ---------- emission schedule ----------
            def mk(fn, *a):
                return lambda: fn(*a)

            # startup: enough of Q/K(g0) to begin head 0
            for s in range(2):
                for half in range(2):
                    q_piece(0, s, half)
            for s in range(2):
                for half in range(2):
                    k_piece(0, 0, s, half)

            work = []
            work += [mk(k_piece, 0, tch, s, half)
                     for tch in range(1, 4) for s in range(2) for half in range(2)]
            work += [mk(v_piece, 0, i) for i in range(16)]
            work += [mk(k_piece, 1, tch, s, half)
                     for tch in range(4) for s in range(2) for half in range(2)]
            work += [mk(q_piece, 1, s, half) for s in range(2) for half in range(2)]
            work += [mk(v_piece, 1, i) for i in range(16)]
            work += [mk(hp_transpose, 0)]
            work.reverse()   # pop() from the end
            heads_extra = {5: mk(hp_transpose, 1), 7: mk(hp_transpose, 2)}

            A_cur = scores_head(0, {(2 * i + 1): work.pop()
                                    for i in range(16) if work})
            for h in range(H):
                if h + 1 < H:
                    ex = {}
                    Ah = A_cur
                    if h + 1 in heads_extra:
                        ex[1] = heads_extra[h + 1]
                    for i in range(16):
                        if i == 12:
                            ex[2 * i + 1] = mk(av_half, h, Ah, 0)
                        elif i == 14:
                            ex[2 * i + 1] = mk(av_half, h, Ah, 1)
                        elif (2 * i + 1) not in ex and work:
                            ex[2 * i + 1] = work.pop()
                    A_cur = scores_head(h + 1, ex)
                else:
                    av_half(h, A_cur, 0)
                    av_half(h, A_cur, 1)
                    hp_transpose(3, halves=(0,))
                    tail_pair(0)
                    hp_transpose(3, halves=(1,))
                    tail_pair(1)
                    tail_pair(2)
                    tail_pair(3)

    nc.compile()
    return nc


def _prep_consts(Wq, bq, Wk, bk, Wv, bv, Wo, bo):
    # wk/wq: [p, ct, g, s, c0]; D = (g*4 + c0//32)*64 + s*32 + c0%32
    def fold_w(W):
        wf = np.empty((128, CT, 2, 2, 128), np.float32)
        c0 = np.arange(128)
        for g in range(2):
            for s in range(2):
                D = (g * 4 + c0 // 32) * 64 + s * 32 + c0 % 32
                for ct in range(CT):
                    wf[:, ct, g, s, :] = W[D, ct * 128:(ct + 1) * 128].T
        return wf.reshape(128, CT * 512).astype(bft)

    wkb = fold_w(Wk)
    wqb = fold_w(0.5 * Wq)
    # wv: [p, ct, col = g*256 + hi*64 + d] = Wv[(g*4+hi)*64+d, ct*128+p]
    wvb = np.ascontiguousarray(
        Wv.T.reshape(CT, 128, 512).transpose(1, 0, 2)
    ).reshape(128, CT * 512).astype(bft)
    # wo: [p, c, C0] = Wo[C0, c*128+p]
    wob = np.ascontiguousarray(
        Wo.T.reshape(4, 128, 512).transpose(1, 0, 2)
    ).reshape(128, 4 * 512).astype(bft)
    # bq: [p, g*2+s] = 0.5*bq[(g*4+p//32)*64 + s*32 + p%32]
    bqa = np.empty((128, 4), np.float32)
    p = np.arange(128)
    for g in range(2):
        for s in range(2):
            bqa[:, g * 2 + s] = 0.5 * bq[(g * 4 + p // 32) * 64 + s * 32 + p % 32]
    bo1 = (bo + Wo @ bv).reshape(1, 512).astype(np.float32)
    one1 = np.ones((1, 128), np.float32)
    idb = np.eye(128, dtype=np.float32).astype(bft)
    return {"wkb": wkb, "wqb": wqb, "wvb": wvb, "wob": wob,
            "bqs": bqa, "bo1": bo1, "one1": one1, "idb": idb}


def kernel(x, Wq, bq, Wk, bk, Wv, bv, Wo, bo):
    x = np.asarray(x, np.float32)
    consts = _prep_consts(*[np.asarray(a, np.float32) for a in
                            (Wq, bq, Wk, bk, Wv, bv, Wo, bo)])
    if "nc" not in _cache:
        _cache["nc"] = _build()
    nc = _cache["nc"]
    in_maps = []
    for i in range(8):
        b, q0 = i // 4, (i % 4) * TQ
        xbT = np.ascontiguousarray(np.roll(x[b], -q0, axis=0).T).astype(bft)
        m = {"xbT": xbT}
        m.update(consts)
        in_maps.append(m)
    res = run_bass_kernel_spmd(nc, in_maps, list(range(8)))
    outf = np.empty((B, T, C), np.float32)
    for i in range(8):
        b, q0 = i // 4, (i % 4) * TQ
        outf[b, q0:q0 + TQ, :] = res.results[i]["out"]
    return outf


# revision 5
# speedup vs baseline: 1.0308x; 1.0026x over previous
"""Multi-head attention (B=2, T=4096, D=512, H=8) on 8 Trainium2 cores — v2.

Sharding: core i = batch i//4, query rows (i%4)*1024..+1024, all heads.
Host rolls x along T so each core's queries sit at t=0:1024.

Engine plan (per core):
- PE: bf16 projections (Q,K,V,O); fp8e4m3 DoubleRow for scores (d_k split
  32+32 into DR subtiles, K/Q stored "folded" [32p, 2, t] via host-permuted
  weight columns) and for attn@V (k-tile pairs are natural DR subtiles).
- Softmax exp is the bottleneck: split across ACT (true Exp -> fp8 out) and
  DVE (Schraudolph bit-trick: round(a*s+b) as int8, bitcast to fp8e4m3).
- Rowsum via ones column in V; normalize on DVE with stride-0 broadcast.
- attn^T via PE transpose (bf16, identity), O proj bf16, bias via 1-row
  matmul. b_k dropped (softmax-invariant), b_v folded into b_o host-side.
Emission interleaves A@V halves and g1 projections inside the next head's
scores/exp stream so the in-order engine queues never starve.
"""
import sys
sys.path.insert(0, "/opt/trn_rl_repo")

import numpy as np
import ml_dtypes
import concourse.bacc as bacc
import concourse.mybir as mybir
import concourse.tile as tile
from concourse.bass_utils import run_bass_kernel_spmd

F32 = mybir.dt.float32
BF16 = mybir.dt.bfloat16
F8 = mybir.dt.float8e4
I8 = mybir.dt.int8
F32R = mybir.dt.float32r
AF = mybir.ActivationFunctionType
ALU = mybir.AluOpType
DR = mybir.MatmulPerfMode.DoubleRow

f8t = ml_dtypes.float8_e4m3
bft = ml_dtypes.bfloat16

B, T, C = 2, 4096, 512
H, DK, TQ = 8, 64, 1024
CT, KT, QT = 4, 32, 8

EXP_SCALE = 0.25                      # q scaled x4 host-side; /8 net
SCH_A = (8.0 / np.log(2.0)) * EXP_SCALE
SCH_B = 56.0                          # hw rounds half-away; bias 7*8
ACT_N = 17                            # exp tiles on ACT per 32

_cache = {}


def _build():
    nc = bacc.Bacc("TRN2")
    xbT = nc.declare_dram_parameter("xbT", [C, T], BF16, isOutput=False)
    wkb = nc.declare_dram_parameter("wkb", [128, CT * 512], BF16, isOutput=False)
    wqb = nc.declare_dram_parameter("wqb", [128, CT * 512], BF16, isOutput=False)
    wvb = nc.declare_dram_parameter("wvb", [128, CT * 512], BF16, isOutput=False)
    wob = nc.declare_dram_parameter("wob", [128, 4 * 512], BF16, isOutput=False)
    bqs = nc.declare_dram_parameter("bqs", [128, 4], F32, isOutput=False)
    bo1 = nc.declare_dram_parameter("bo1", [1, 512], F32R, isOutput=False)
    one1 = nc.declare_dram_parameter("one1", [1, 128], F32R, isOutput=False)
    idb = nc.declare_dram_parameter("idb", [128, 128], BF16, isOutput=False)
    out = nc.declare_dram_parameter("out", [TQ, C], F32, isOutput=True)

    cvt_toggle = [0]
    ob_toggle = [0]

    with tile.TileContext(nc) as tc:
        with (
            tc.tile_pool(name="const", bufs=1) as cpool,
            tc.tile_pool(name="sb", bufs=1) as sb,
            tc.tile_pool(name="apool", bufs=2) as apool,
            tc.tile_pool(name="atp", bufs=2) as atp,
            tc.tile_pool(name="outp", bufs=2) as outp,
            tc.tile_pool(name="rp", bufs=2) as rp,
            tc.tile_pool(name="big", bufs=3, space="PSUM") as big,
            tc.tile_pool(name="small", bufs=2, space="PSUM") as small,
        ):
            # ---------- x queries chunk + early consts first ----------
            xT = sb.tile([128, CT, T], BF16, tag="xT")
            for ct in range(CT):
                nc.sync.dma_start(xT[:, ct, 0:1024], xbT[ct * 128:(ct + 1) * 128, 0:1024])
            wq = cpool.tile([128, CT, 2, 2, 128], BF16, tag="wq")
            nc.scalar.dma_start(
                wq[:], wqb.rearrange("p (a b c d) -> p a b c d", a=CT, b=2, c=2))
            wk = cpool.tile([128, CT, 2, 2, 128], BF16, tag="wk")
            nc.scalar.dma_start(
                wk[:], wkb.rearrange("p (a b c d) -> p a b c d", a=CT, b=2, c=2))
            bq = cpool.tile([128, 4], F32, tag="bq")
            nc.scalar.dma_start(bq[:], bqs[:])
            wv = cpool.tile([128, CT, 512], BF16, tag="wv")
            nc.sync.dma_start(wv[:], wvb.rearrange("p (a b) -> p a b", a=CT))
            ident = cpool.tile([128, 128], BF16, tag="ident")
            nc.sync.dma_start(ident[:], idb[:])
            ones = cpool.tile([1, 128], F32R, tag="ones")
            nc.sync.dma_start(ones[:], one1[:])
            bos = cpool.tile([1, 512], F32R, tag="bos")
            nc.sync.dma_start(bos[:], bo1[:])
            wo = cpool.tile([128, 4, 512], BF16, tag="wo")
            nc.sync.dma_start(wo[:], wob.rearrange("p (a b) -> p a b", a=4))
            for tch in range(1, 4):
                for ct in range(CT):
                    nc.sync.dma_start(
                        xT[:, ct, tch * 1024:(tch + 1) * 1024],
                        xbT[ct * 128:(ct + 1) * 128, tch * 1024:(tch + 1) * 1024])

            # folded fp8 K^T/Q^T: [p = 4heads x 32dsub, g, plane, t]
            kfold = sb.tile([128, 2, 2, T], F8, tag="kfold")
            qfold = sb.tile([128, 2, 2, TQ], F8, tag="qfold")
            # V: [p=k-within-tile, d(65: row 64=ones), j, g, hi]
            vsb = sb.tile([128, 65, KT, 2, 4], F8, tag="vsb")
            nc.vector.memset(vsb[:, 64, :, :, :], 1.0)
            # normalized attention, [p=q-within-tile, qt, h, d]
            attn = sb.tile([128, QT, H, DK], BF16, tag="attn")

            # ---------- converts (psum f32 -> sbuf fp8) ----------
            def convert(dst, src, bias=None):
                eng = 0 if cvt_toggle[0] % 2 == 0 else 1   # 1/2 on ACT
                cvt_toggle[0] += 1
                if eng == 0:
                    if bias is not None:
                        nc.scalar.activation(dst, src, AF.Identity, bias=bias)
                    else:
                        nc.scalar.copy(dst, src)
                else:
                    if bias is not None:
                        nc.vector.tensor_scalar_add(dst, src, bias)
                    else:
                        nc.vector.tensor_copy(dst, src)

            # ---------- projections (512-wide pieces, small psum ring) ----------
            def q_piece(g, s, half):
                pq = small.tile([128, 512], F32, tag="sm")
                for ct in range(CT):
                    nc.tensor.matmul(
                        pq[:], wq[:, ct, g, s, :],
                        xT[:, ct, half * 512:(half + 1) * 512],
                        start=(ct == 0), stop=(ct == CT - 1))
                convert(qfold[:, g, s, half * 512:(half + 1) * 512], pq[:],
                        bias=bq[:, g * 2 + s:g * 2 + s + 1])

            def k_piece(g, tch, s, half):
                pk = small.tile([128, 512], F32, tag="sm")
                c0 = tch * 1024 + half * 512
                for ct in range(CT):
                    nc.tensor.matmul(
                        pk[:], wk[:, ct, g, s, :], xT[:, ct, c0:c0 + 512],
                        start=(ct == 0), stop=(ct == CT - 1))
                convert(kfold[:, g, s, c0:c0 + 512], pk[:])

            def v_piece(g, jq2):
                # two k-tiles (j = 2*jq2, +1)
                pv = small.tile([128, 2, 256], F32, tag="sm")
                for jj in range(2):
                    j = jq2 * 2 + jj
                    for ct in range(CT):
                        nc.tensor.matmul(
                            pv[:, jj, :],
                            xT[:, ct, j * 128:(j + 1) * 128],
                            wv[:, ct, g * 256:(g + 1) * 256],
                            start=(ct == 0), stop=(ct == CT - 1))
                convert(
                    vsb[:, 0:64, jq2 * 2:(jq2 + 1) * 2, g, :],
                    pv[:].rearrange("p a (c d) -> p d a c", c=4))

            # ---------- attention ----------
            def scores_head(h, extras):
                g, hi = divmod(h, 4)
                base = 32 * hi
                A = apool.tile([128, KT, TQ], F8, tag="A")
                for j in range(KT):
                    pss = big.tile([128, 1024], F32, tag="pp")
                    for half in range(2):
                        nc.tensor.matmul(
                            pss[:, half * 512:(half + 1) * 512],
                            kfold[base:base + 32, g, :, j * 128:(j + 1) * 128],
                            qfold[base:base + 32, g, :, half * 512:(half + 1) * 512],
                            start=True, stop=True, perf_mode=DR,
                            tile_position=(base, 0))
                    n_act = ACT_N + 1 if h >= 2 else ACT_N
                    if (j * n_act) % 32 < n_act:
                        nc.scalar.activation(A[:, j, :], pss[:], AF.Exp,
                                             scale=EXP_SCALE)
                    else:
                        nc.vector.tensor_scalar(A[:, j, :].bitcast(I8), pss[:],
                                                SCH_A, SCH_B, ALU.mult, ALU.add)
                    if j in extras:
                        extras[j]()
                return A

            def av_half(h, A, qh):
                g, hi = divmod(h, 4)
                av = small.tile([128, 4, 65], F32, tag="sm")
                for qq in range(4):
                    qt = qh * 4 + qq
                    for jp in range(16):
                        nc.tensor.matmul(
                            av[:, qq, :],
                            A[:, 2 * jp:2 * jp + 2, qt * 128:(qt + 1) * 128],
                            vsb[:, :, 2 * jp:2 * jp + 2, g, hi].rearrange(
                                "p d a -> p a d"),
                            start=(jp == 0), stop=(jp == 15), perf_mode=DR)
                ri = rp.tile([128, 4], F32, tag="ri")
                nc.vector.reciprocal(ri[:], av[:, :, 64])
                nc.vector.tensor_tensor(
                    out=attn[:, qh * 4:(qh + 1) * 4, h, :],
                    in0=av[:, :, 0:64],
                    in1=ri[:][:, :, None].broadcast_to([128, 4, 64]),
                    op=ALU.mult)

            # ---------- output projection ----------
            # attn^T accumulates in SBUF per head-pair as soon as both heads
            # are normalized; the tail is only the last pair + O-proj chains.
            aTsb = sb.tile([128, 4, QT, 128], BF16, tag="aTsb")

            def hp_transpose(c, halves=(0, 1)):
                for hf in halves:
                    pt = small.tile([128, 4, 128], BF16, tag="sm")
                    for qq in range(4):
                        qt = hf * 4 + qq
                        nc.tensor.matmul(
                            pt[:, qq, :],
                            attn[:, qt, 2 * c:2 * c + 2, :],
                            ident[:],
                            is_transpose=True)
                    nc.vector.tensor_copy(aTsb[:, c, hf * 4:(hf + 1) * 4, :], pt[:])

            def tail_pair(pair):
                po = big.tile([128, 2, 512], F32, tag="pp")
                for i in range(2):
                    qt = 2 * pair + i
                    for c in range(4):
                        nc.tensor.matmul(po[:, i, :], aTsb[:, c, qt, :],
                                         wo[:, c, :],
                                         start=(c == 0), stop=False)
                    nc.tensor.matmul(po[:, i, :], ones[:], bos[:],
                                     start=False, stop=True)
                ob = outp.tile([128, 2, 512], F32, tag="ob")
                if ob_toggle[0] % 2 == 0:
                    nc.scalar.copy(ob[:], po[:])
                else:
                    nc.vector.tensor_copy(ob[:], po[:])
                ob_toggle[0] += 1
                for i in range(2):
                    qt = 2 * pair + i
                    nc.sync.dma_start(out[qt * 128:(qt + 1) * 128, :],
                                      ob[:, i, :])

            # ---------- emission schedule ----------
            def mk(fn, *a):
                return lambda: fn(*a)

            # startup: enough of Q/K(g0) to begin head 0
            for s in range(2):
                for half in range(2):
                    q_piece(0, s, half)
            for s in range(2):
                for half in range(2):
                    k_piece(0, 0, s, half)

            work = []
            work += [mk(k_piece, 0, tch, s, half)
                     for tch in range(1, 4) for s in range(2) for half in range(2)]
            work += [mk(v_piece, 0, i) for i in range(16)]
            work += [mk(k_piece, 1, tch, s, half)
                     for tch in range(4) for s in range(2) for half in range(2)]
            work += [mk(q_piece, 1, s, half) for s in range(2) for half in range(2)]
            work += [mk(v_piece, 1, i) for i in range(16)]
            work += [mk(hp_transpose, 0)]
            work.reverse()   # pop() from the end
            heads_extra = {5: mk(hp_transpose, 1), 7: mk(hp_transpose, 2)}

            A_cur = scores_head(0, {(2 * i + 1): work.pop()
                                    for i in range(16) if work})
            for h in range(H):
                if h + 1 < H:
                    ex = {}
                    Ah = A_cur
                    if h + 1 in heads_extra:
                        ex[1] = heads_extra[h + 1]
                    for i in range(16):
                        if i == 12:
                            ex[2 * i + 1] = mk(av_half, h, Ah, 0)
                        elif i == 14:
                            ex[2 * i + 1] = mk(av_half, h, Ah, 1)
                        elif (2 * i + 1) not in ex and work:
                            ex[2 * i + 1] = work.pop()
                    A_cur = scores_head(h + 1, ex)
                else:
                    av_half(h, A_cur, 0)
                    av_half(h, A_cur, 1)
                    hp_transpose(3, halves=(0,))
                    tail_pair(0)
                    hp_transpose(3, halves=(1,))
                    tail_pair(1)
                    tail_pair(2)
                    tail_pair(3)

    nc.compile()
    return nc


def _prep_consts(Wq, bq, Wk, bk, Wv, bv, Wo, bo):
    # wk/wq: [p, ct, g, s, c0]; D = (g*4 + c0//32)*64 + s*32 + c0%32
    def fold_w(W):
        wf = np.empty((128, CT, 2, 2, 128), np.float32)
        c0 = np.arange(128)
        for g in range(2):
            for s in range(2):
                D = (g * 4 + c0 // 32) * 64 + s * 32 + c0 % 32
                for ct in range(CT):
                    wf[:, ct, g, s, :] = W[D, ct * 128:(ct + 1) * 128].T
        return wf.reshape(128, CT * 512).astype(bft)

    wkb = fold_w(Wk)
    wqb = fold_w(0.5 * Wq)
    # wv: [p, ct, col = g*256 + hi*64 + d] = Wv[(g*4+hi)*64+d, ct*128+p]
    wvb = np.ascontiguousarray(
        Wv.T.reshape(CT, 128, 512).transpose(1, 0, 2)
    ).reshape(128, CT * 512).astype(bft)
    # wo: [p, c, C0] = Wo[C0, c*128+p]
    wob = np.ascontiguousarray(
        Wo.T.reshape(4, 128, 512).transpose(1, 0, 2)
    ).reshape(128, 4 * 512).astype(bft)
    # bq: [p, g*2+s] = 0.5*bq[(g*4+p//32)*64 + s*32 + p%32]
    bqa = np.empty((128, 4), np.float32)
    p = np.arange(128)
    for g in range(2):
        for s in range(2):
            bqa[:, g * 2 + s] = 0.5 * bq[(g * 4 + p // 32) * 64 + s * 32 + p % 32]
    bo1 = (bo + Wo @ bv).reshape(1, 512).astype(np.float32)
    one1 = np.ones((1, 128), np.float32)
    idb = np.eye(128, dtype=np.float32).astype(bft)
    return {"wkb": wkb, "wqb": wqb, "wvb": wvb, "wob": wob,
            "bqs": bqa, "bo1": bo1, "one1": one1, "idb": idb}


def kernel(x, Wq, bq, Wk, bk, Wv, bv, Wo, bo):
    x = np.asarray(x, np.float32)
    consts = _prep_consts(*[np.asarray(a, np.float32) for a in
                            (Wq, bq, Wk, bk, Wv, bv, Wo, bo)])
    if "nc" not in _cache:
        _cache["nc"] = _build()
    nc = _cache["nc"]
    in_maps = []
    for i in range(8):
        b, q0 = i // 4, (i % 4) * TQ
        xbT = np.ascontiguousarray(np.roll(x[b], -q0, axis=0).T).astype(bft)
        m = {"xbT": xbT}
        m.update(consts)
        in_maps.append(m)
    res = run_bass_kernel_spmd(nc, in_maps, list(range(8)))
    outf = np.empty((B, T, C), np.float32)
    for i in range(8):
        b, q0 = i // 4, (i % 4) * TQ
        outf[b, q0:q0 + TQ, :] = res.results[i]["out"]
    return outf
